# revision 21
# baseline (speedup 1.0000x reference)
"""KAN-attention Trainium2 kernel (8 NeuronCores, SPMD), fp8 DoubleRow version.

Math per batch b:
    q = x Wq^T + bq ; k = x Wk^T + bk ; v = x Wv^T
    kq = q basis^T ; kk = k basis^T            (rank-16)
    out = softmax(kq kk^T / 32) v + bv

Folding: kq = x Bq^T + cq with Bq = basis Wq (host).  Writing e = exp(l)
= 1 + delta, the attention numerator splits as e@v = colsum(v) + delta@v
where colsum(v) is computed EXACTLY on the host (tiny matvec).  The
device only computes p = delta@v and r = rowsum(delta); fp8 quantization
error is then suppressed by |delta| ~ 0.04, so all heavy matmuls run in
fp8e4m3 with DoubleRow (2 contraction rows per PE cell -> 4x fewer PE
cycles than fp32).

Sharding: core c = 2b+h handles batch b and key-half h (1024 of 2048
keys), sequence rotated on host so keys sit at cols 0:1024 of xt.
Host combine: out_b = (p0+p1 + c0+c1) / (2048 + r0+r1) + bv.

Device dataflow (per core), everything fp8 except where noted:
  kan:    psq[16,2048]  = sum_g Bq8[128,2,16].T @ xt[128,2,512]   (DR)
          kanq[16,2048] (bf16) = psq + cq      (ACT, bias)
  v:      psv[128,512]  = sum_g xt[128,2,128k].T @ wvt[128,2,512] (DR)
          v8[128,(g,t),1024] (fp8)             (DVE copy)
  logits: psl[128k,512q] = kank[16,128k].T @ kanq[16,512q]  (bf16, K=16)
  exp:    e[128,512] f32 = Exp(psl * 2^-15)    (ACT)
  delta:  d8[128,(kc),2048] = e - 1 -> fp8     (DVE/Pool)
  attn:   pso[128q,1024e] += d8[128,2,128q].T @ v8[128,2,512e]    (DR)
  rowsum: psr[1,512] += ones[128,2,1].T @ d8[128,2,512q]          (DR)
  out:    p bf16 via engine copy + DMA; r f32.
"""

import os
import sys

sys.path.insert(0, "/opt/trn_rl_repo")

import math

import numpy as np

DIM = 1024
SEQ = 2048
NF = 16
NCORES = 8
MH = 1024  # keys per core

_cache = {}


def _build():
    import concourse.bass as bass
    import concourse.tile as tile
    from concourse import bacc, mybir

    dt = mybir.dt
    f8 = dt.float8e4
    bf16 = dt.bfloat16
    f32 = dt.float32
    DR = mybir.MatmulPerfMode.DoubleRow
    EXPS = 1.0 / 32768.0  # softmax scale 1/32 / (SB*SB) with SB=32

    nc = bacc.Bacc("TRN2", target_bir_lowering=False)

    xt = nc.declare_dram_parameter("xt", [DIM, SEQ], f8, isOutput=False)
    wvt = nc.declare_dram_parameter("wvt", [DIM, DIM], f8, isOutput=False)
    bqk = nc.declare_dram_parameter("bqk", [DIM, 32], f8, isOutput=False)
    cqk = nc.declare_dram_parameter("cqk", [NF, 2], f32, isOutput=False)
    p_out = nc.declare_dram_parameter("p", [SEQ, DIM], bf16, isOutput=True)
    r_out = nc.declare_dram_parameter("r", [1, SEQ], f32, isOutput=True)

    xt_r = xt.rearrange("(o p) l -> p o l", p=128)    # (128, 8, 2048), o=(g,t)
    wvt_r = wvt.rearrange("(o p) e -> p o e", p=128)  # (128, 8, 1024)
    bqk_r = bqk.rearrange("(o p) f -> p o f", p=128)  # (128, 8, 32)

    with tile.TileContext(nc) as tc:
        with tc.tile_pool(name="res", bufs=1) as res:
            xt_sb = res.tile([128, 8, SEQ], f8)
            wvt_sb = res.tile([128, 8, DIM], f8)
            bqk_sb = res.tile([128, 8, 32], f8)
            cqk_sb = res.tile([NF, 2], f32)
            # [128, 2, 16] so the DoubleRow ldweights k-slot stride (16 B)
            # satisfies the ISA step%16==0 constraint; only [:, :, 0:1] is used
            ones_sb = res.tile([128, 2, 16], f8)
            kanq_sb = res.tile([NF, SEQ], bf16)
            kank_sb = res.tile([NF, MH], bf16)
            v_sb = res.tile([128, 4, 2, DIM], f8)     # keys (g,t) on dims 1,2
            d_sb = res.tile([128, 8, SEQ], f8)        # delta^T, dim1 = key chunk
            r_sb = res.tile([1, SEQ], f32)

            nc.vector.memset(ones_sb, 1.0)
            warm_sb = res.tile([1, 8], f32)
            nc.vector.memset(warm_sb, 0.0)
            nc.scalar.activation(
                out=warm_sb, in_=warm_sb,
                func=mybir.ActivationFunctionType.Exp, scale=1.0,
            )

            # input DMAs: key-half of xt + wvt first so the v matmuls can
            # start early; query half streams in behind them
            nc.sync.dma_start(out=bqk_sb[:], in_=bqk_r[:])
            nc.sync.dma_start(out=xt_sb[:, :, 0:512], in_=xt_r[:, :, 0:512])
            nc.sync.dma_start(out=cqk_sb[:], in_=cqk[:])
            nc.sync.dma_start(out=wvt_sb[:, 0:4, :], in_=wvt_r[:, 0:4, :])
            nc.sync.dma_start(out=xt_sb[:, :, 512:MH], in_=xt_r[:, :, 512:MH])
            nc.sync.dma_start(out=wvt_sb[:, 4:8, :], in_=wvt_r[:, 4:8, :])
            nc.sync.dma_start(out=xt_sb[:, :, MH:SEQ], in_=xt_r[:, :, MH:SEQ])

            with (
                tc.tile_pool(name="psl", bufs=2, space="PSUM") as pslp,
                tc.tile_pool(name="ep", bufs=8) as ep,
                tc.tile_pool(name="pp", bufs=4) as pp,
            ):
                ncopy = {"i": 0}

                def kan_group(dst, col0, w, bias, tag):
                    ps = pskan.tile([NF, 512], f32, name="pskan_t")
                    for g in range(4):
                        nc.tensor.matmul(
                            ps[:, 0:w],
                            bqk_sb[:, 2 * g:2 * g + 2, tag],
                            xt_sb[:, 2 * g:2 * g + 2, col0:col0 + w],
                            start=(g == 0), stop=(g == 3), perf_mode=DR,
                        )
                    nc.scalar.activation(
                        out=dst[:, col0:col0 + w], in_=ps[:, 0:w],
                        func=mybir.ActivationFunctionType.Identity,
                        bias=bias, scale=1.0,
                    )

                def logits_mc(qc, mc):
                    qs = slice(qc * 512, (qc + 1) * 512)
                    pl = pslp.tile([128, 512], f32, name="psl_t")
                    nc.tensor.matmul(
                        pl,
                        kank_sb[:, mc * 128:(mc + 1) * 128],
                        kanq_sb[:, qs],
                        start=True, stop=True,
                    )
                    et = ep.tile([128, 512], f32, name="ep_t")
                    nc.scalar.activation(
                        out=et, in_=pl,
                        func=mybir.ActivationFunctionType.Exp,
                        scale=EXPS,
                    )
                    i = qc * 8 + mc
                    if (qc < 2 and i % 4 == 3) or (qc >= 2 and i % 2 == 0):
                        eng = nc.vector
                    else:
                        eng = nc.gpsimd
                    eng.tensor_scalar_sub(
                        out=d_sb[:, mc, qs], in0=et, scalar1=1.0,
                    )

                def attn_qc(qc, split=False):
                    po = psop.tile([128, DIM], f32, name="pso_t")
                    for g in range(4):
                        for eh in range(2):
                            nc.tensor.matmul(
                                po[:, eh * 512:(eh + 1) * 512],
                                d_sb[:, 2 * g:2 * g + 2, qc * 128:(qc + 1) * 128],
                                v_sb[:, g, :, eh * 512:(eh + 1) * 512],
                                start=(g == 0), stop=(g == 3), perf_mode=DR,
                            )
                    pt = pp.tile([128, DIM], bf16, name="pp_t")
                    if split:
                        # tail latency: halve the copy across both engines
                        nc.vector.tensor_copy(out=pt[:, 0:512], in_=po[:, 0:512])
                        nc.scalar.copy(out=pt[:, 512:DIM], in_=po[:, 512:DIM])
                    else:
                        i = ncopy["i"]
                        if i in (1, 5, 9, 10, 11, 12):
                            nc.scalar.copy(out=pt[:], in_=po)
                        else:
                            nc.vector.tensor_copy(out=pt[:], in_=po)
                    ncopy["i"] += 1
                    nc.sync.dma_start(
                        out=p_out[qc * 128:(qc + 1) * 128, :], in_=pt[:]
                    )

                def rowsum(g4):
                    qs = slice(g4 * 512, (g4 + 1) * 512)
                    psr = pslp.tile([128, 512], f32, name="psl_t")
                    for g in range(4):
                        nc.tensor.matmul(
                            psr[0:1, :],
                            ones_sb[:, :, 0:1],
                            d_sb[:, 2 * g:2 * g + 2, qs],
                            start=(g == 0), stop=(g == 3), perf_mode=DR,
                        )
                    nc.vector.tensor_copy(out=r_sb[:, qs], in_=psr[0:1, :])

                with tc.tile_pool(name="pskan", bufs=2, space="PSUM") as pskan:
                    with tc.tile_pool(name="psv", bufs=2, space="PSUM") as psv:
                        vps = {}

                        def v_mms(kc, gr):
                            if kc not in vps:
                                vps[kc] = psv.tile([128, DIM], f32, name="psv_t")
                            ps = vps[kc]
                            for g in gr:
                                for eh in range(2):
                                    nc.tensor.matmul(
                                        ps[:, eh * 512:(eh + 1) * 512],
                                        xt_sb[:, 2 * g:2 * g + 2, kc * 128:(kc + 1) * 128],
                                        wvt_sb[:, 2 * g:2 * g + 2, eh * 512:(eh + 1) * 512],
                                        start=(g == 0), stop=(g == 3), perf_mode=DR,
                                    )
                            if gr[-1] == 3:
                                if kc % 4 == 3:
                                    nc.scalar.copy(out=v_sb[:, kc // 2, kc % 2, :], in_=vps[kc])
                                else:
                                    nc.vector.tensor_copy(
                                        out=v_sb[:, kc // 2, kc % 2, :], in_=vps[kc]
                                    )
                                del vps[kc]

                        # schedule around DMA arrival: xt keys -> wvt half ->
                        # xt keys 2nd half -> wvt 2nd half -> xt queries
                        kan_group(kanq_sb, 0, 512, cqk_sb[:, 0:1], slice(0, NF))
                        kan_group(kank_sb, 0, 512, cqk_sb[:, 1:2], slice(NF, 32))
                        v_mms(0, [0, 1])
                        v_mms(1, [0, 1])
                        kan_group(kank_sb, 512, 512, cqk_sb[:, 1:2], slice(NF, 32))
                        kan_group(kanq_sb, 512, 512, cqk_sb[:, 0:1], slice(0, NF))
                        v_mms(0, [2, 3])
                        v_mms(1, [2, 3])

                        for mc in range(8):
                            logits_mc(0, mc)
                        for kc in range(2, 8):
                            v_mms(kc, [0, 1, 2, 3])
                        kan_group(kanq_sb, 1024, 512, cqk_sb[:, 0:1], slice(0, NF))
                        kan_group(kanq_sb, 1536, 512, cqk_sb[:, 0:1], slice(0, NF))
                        for mc in range(8):
                            logits_mc(1, mc)

                with tc.tile_pool(name="pso", bufs=3, space="PSUM") as psop:
                    # fine interleave: attn group g with logits group g+2
                    attn_qc(0); logits_mc(2, 0); logits_mc(2, 1)
                    attn_qc(1); logits_mc(2, 2); logits_mc(2, 3)
                    attn_qc(2); logits_mc(2, 4); logits_mc(2, 5)
                    attn_qc(3); logits_mc(2, 6); logits_mc(2, 7)
                    attn_qc(4); rowsum(0)
                    attn_qc(5); logits_mc(3, 0); logits_mc(3, 1)
                    attn_qc(6); logits_mc(3, 2); logits_mc(3, 3)
                    attn_qc(7); logits_mc(3, 4); logits_mc(3, 5)
                    attn_qc(8); logits_mc(3, 6); logits_mc(3, 7)
                    attn_qc(9); rowsum(1)
                    attn_qc(10); attn_qc(11); rowsum(2)
                    attn_qc(12)
                    rowsum(3)
                    nc.sync.dma_start(out=r_out[:], in_=r_sb[:])
                    attn_qc(13, split=True)
                    attn_qc(14, split=True)
                    attn_qc(15, split=True)

    nc.compile()
    return nc


def _get_nc():
    if "nc" not in _cache:
        _cache["nc"] = _build()
    return _cache["nc"]


def kernel(x, basis, Wq, bq, Wk, bk, Wv, bv, _trace=False):
    import ml_dtypes
    from concourse.bass_utils import run_bass_kernel_spmd

    f8 = ml_dtypes.float8_e4m3

    x = np.asarray(x, dtype=np.float32)
    basis = np.asarray(basis, dtype=np.float32)
    Wq = np.asarray(Wq, dtype=np.float32)
    bq = np.asarray(bq, dtype=np.float32)
    Wk = np.asarray(Wk, dtype=np.float32)
    bk = np.asarray(bk, dtype=np.float32)
    Wv = np.asarray(Wv, dtype=np.float32)
    bv = np.asarray(bv, dtype=np.float32)

    SB = np.float32(32.0)
    Bq = (basis @ Wq) * SB            # (16, 1024); exp scale 2^-15 on device
    Bk = (basis @ Wk) * SB
    cq = (basis @ bq) * SB
    ck = (basis @ bk) * SB
    bqk_np = np.zeros((DIM, 32), dtype=np.float32)
    bqk_np[:, 0:NF] = Bq.T
    bqk_np[:, NF:32] = Bk.T
    bqk_np = bqk_np.astype(f8)
    cqk_np = np.stack([cq, ck], axis=1).astype(np.float32)  # (16, 2)
    wvt_np = np.ascontiguousarray(Wv.T).astype(f8)          # (din, e)

    nc = _get_nc()
    in_maps = []
    for c in range(NCORES):
        b, h = c // 2, c % 2
        xtb = x[b].T  # (1024, 2048)
        if h == 1:
            xtb = np.concatenate([xtb[:, MH:], xtb[:, :MH]], axis=1)
        in_maps.append(
            {
                "xt": np.ascontiguousarray(xtb).astype(f8),
                "wvt": wvt_np,
                "bqk": bqk_np,
                "cqk": cqk_np,
            }
        )

    res = run_bass_kernel_spmd(nc, in_maps, list(range(NCORES)), trace=_trace)
    kernel.last_results = res

    # exact colsum-of-v correction on host: c_half = (sum over keys of x) @ Wv.T
    out = np.empty((4, SEQ, DIM), dtype=np.float32)
    for b in range(4):
        c0 = (x[b, :MH, :].sum(axis=0, dtype=np.float64) @ Wv.T.astype(np.float64))
        c1 = (x[b, MH:, :].sum(axis=0, dtype=np.float64) @ Wv.T.astype(np.float64))
        p0 = res.results[2 * b]["p"].astype(np.float32)
        p1 = res.results[2 * b + 1]["p"].astype(np.float32)
        r0 = res.results[2 * b]["r"][0]
        r1 = res.results[2 * b + 1]["r"][0]
        p1 = np.roll(p1, MH, axis=0)
        r1 = np.roll(r1, MH, axis=0)
        num = p0 + p1 + (c0 + c1).astype(np.float32)[None, :]
        den = np.float32(SEQ) + r0 + r1
        out[b] = num / den[:, None] + bv
    return out


# revision 22
# speedup vs baseline: 1.0070x; 1.0070x over previous
"""KAN-attention Trainium2 kernel (8 NeuronCores, SPMD), fp8 DoubleRow version.

Math per batch b:
    q = x Wq^T + bq ; k = x Wk^T + bk ; v = x Wv^T
    kq = q basis^T ; kk = k basis^T            (rank-16)
    out = softmax(kq kk^T / 32) v + bv

Folding: kq = x Bq^T + cq with Bq = basis Wq (host).  Writing e = exp(l)
= 1 + delta, the attention numerator splits as e@v = colsum(v) + delta@v
where colsum(v) is computed EXACTLY on the host (tiny matvec).  The
device only computes p = delta@v and r = rowsum(delta); fp8 quantization
error is then suppressed by |delta| ~ 0.04, so all heavy matmuls run in
fp8e4m3 with DoubleRow (2 contraction rows per PE cell -> 4x fewer PE
cycles than fp32).

Sharding: core c = 2b+h handles batch b and key-half h (1024 of 2048
keys), sequence rotated on host so keys sit at cols 0:1024 of xt.
Host combine: out_b = (p0+p1 + c0+c1) / (2048 + r0+r1) + bv.

Device dataflow (per core), everything fp8 except where noted:
  kan:    psq[16,2048]  = sum_g Bq8[128,2,16].T @ xt[128,2,512]   (DR)
          kanq[16,2048] (bf16) = psq + cq      (ACT, bias)
  v:      psv[128,512]  = sum_g xt[128,2,128k].T @ wvt[128,2,512] (DR)
          v8[128,(g,t),1024] (fp8)             (DVE copy)
  logits: psl[128k,512q] = kank[16,128k].T @ kanq[16,512q]  (bf16, K=16)
  exp:    e[128,512] f32 = Exp(psl * 2^-15)    (ACT)
  delta:  d8[128,(kc),2048] = e - 1 -> fp8     (DVE/Pool)
  attn:   pso[128q,1024e] += d8[128,2,128q].T @ v8[128,2,512e]    (DR)
  rowsum: psr[1,512] += ones[128,2,1].T @ d8[128,2,512q]          (DR)
  out:    p bf16 via engine copy + DMA; r f32.
"""

import os
import sys

sys.path.insert(0, "/opt/trn_rl_repo")

import math

import numpy as np

DIM = 1024
SEQ = 2048
NF = 16
NCORES = 8
MH = 1024  # keys per core

_cache = {}


def _build():
    import concourse.bass as bass
    import concourse.tile as tile
    from concourse import bacc, mybir

    dt = mybir.dt
    f8 = dt.float8e4
    bf16 = dt.bfloat16
    f32 = dt.float32
    DR = mybir.MatmulPerfMode.DoubleRow
    EXPS = 1.0 / 32768.0  # softmax scale 1/32 / (SB*SB) with SB=32

    nc = bacc.Bacc("TRN2", target_bir_lowering=False)

    xt = nc.declare_dram_parameter("xt", [DIM, SEQ], f8, isOutput=False)
    wvt = nc.declare_dram_parameter("wvt", [DIM, DIM], f8, isOutput=False)
    bqk = nc.declare_dram_parameter("bqk", [DIM, 32], f8, isOutput=False)
    cqk = nc.declare_dram_parameter("cqk", [NF, 2], f32, isOutput=False)
    p_out = nc.declare_dram_parameter("p", [SEQ, DIM], bf16, isOutput=True)
    r_out = nc.declare_dram_parameter("r", [1, SEQ], f32, isOutput=True)

    xt_r = xt.rearrange("(o p) l -> p o l", p=128)    # (128, 8, 2048), o=(g,t)
    wvt_r = wvt.rearrange("(o p) e -> p o e", p=128)  # (128, 8, 1024)
    bqk_r = bqk.rearrange("(o p) f -> p o f", p=128)  # (128, 8, 32)

    with tile.TileContext(nc) as tc:
        with tc.tile_pool(name="res", bufs=1) as res:
            xt_sb = res.tile([128, 8, SEQ], f8)
            wvt_sb = res.tile([128, 8, DIM], f8)
            bqk_sb = res.tile([128, 8, 32], f8)
            cqk_sb = res.tile([NF, 2], f32)
            # [128, 2, 16] so the DoubleRow ldweights k-slot stride (16 B)
            # satisfies the ISA step%16==0 constraint; only [:, :, 0:1] is used
            ones_sb = res.tile([128, 2, 16], f8)
            kanq_sb = res.tile([NF, SEQ], bf16)
            kank_sb = res.tile([NF, MH], bf16)
            v_sb = res.tile([128, 4, 2, DIM], f8)     # keys (g,t) on dims 1,2
            d_sb = res.tile([128, 8, SEQ], f8)        # delta^T, dim1 = key chunk
            r_sb = res.tile([1, SEQ], f32)

            nc.vector.memset(ones_sb, 1.0)
            warm_sb = res.tile([1, 8], f32)
            nc.vector.memset(warm_sb, 0.0)
            nc.scalar.activation(
                out=warm_sb, in_=warm_sb,
                func=mybir.ActivationFunctionType.Exp, scale=1.0,
            )

            # input DMAs: key-half of xt + wvt first so the v matmuls can
            # start early; query half streams in behind them
            nc.sync.dma_start(out=bqk_sb[:], in_=bqk_r[:])
            nc.sync.dma_start(out=xt_sb[:, :, 0:512], in_=xt_r[:, :, 0:512])
            nc.sync.dma_start(out=cqk_sb[:], in_=cqk[:])
            nc.sync.dma_start(out=wvt_sb[:, 0:4, :], in_=wvt_r[:, 0:4, :])
            nc.sync.dma_start(out=xt_sb[:, :, 512:MH], in_=xt_r[:, :, 512:MH])
            nc.sync.dma_start(out=wvt_sb[:, 4:8, :], in_=wvt_r[:, 4:8, :])
            nc.sync.dma_start(out=xt_sb[:, :, MH:SEQ], in_=xt_r[:, :, MH:SEQ])

            with (
                tc.tile_pool(name="psl", bufs=2, space="PSUM") as pslp,
                tc.tile_pool(name="ep", bufs=8) as ep,
                tc.tile_pool(name="pp", bufs=4) as pp,
            ):
                ncopy = {"i": 0}

                def kan_group(dst, col0, w, bias, tag):
                    ps = pskan.tile([NF, 512], f32, name="pskan_t")
                    for g in range(4):
                        nc.tensor.matmul(
                            ps[:, 0:w],
                            bqk_sb[:, 2 * g:2 * g + 2, tag],
                            xt_sb[:, 2 * g:2 * g + 2, col0:col0 + w],
                            start=(g == 0), stop=(g == 3), perf_mode=DR,
                        )
                    nc.scalar.activation(
                        out=dst[:, col0:col0 + w], in_=ps[:, 0:w],
                        func=mybir.ActivationFunctionType.Identity,
                        bias=bias, scale=1.0,
                    )

                def logits_mc(qc, mc):
                    qs = slice(qc * 512, (qc + 1) * 512)
                    pl = pslp.tile([128, 512], f32, name="psl_t")
                    nc.tensor.matmul(
                        pl,
                        kank_sb[:, mc * 128:(mc + 1) * 128],
                        kanq_sb[:, qs],
                        start=True, stop=True,
                    )
                    et = ep.tile([128, 512], f32, name="ep_t")
                    nc.scalar.activation(
                        out=et, in_=pl,
                        func=mybir.ActivationFunctionType.Exp,
                        scale=EXPS,
                    )
                    i = qc * 8 + mc
                    if (qc < 2 and i % 4 == 3) or (qc >= 2 and i % 2 == 0):
                        eng = nc.vector
                    else:
                        eng = nc.gpsimd
                    eng.tensor_scalar_sub(
                        out=d_sb[:, mc, qs], in0=et, scalar1=1.0,
                    )

                def attn_qc(qc, split=False):
                    po = psop.tile([128, DIM], f32, name="pso_t")
                    for g in range(4):
                        for eh in range(2):
                            nc.tensor.matmul(
                                po[:, eh * 512:(eh + 1) * 512],
                                d_sb[:, 2 * g:2 * g + 2, qc * 128:(qc + 1) * 128],
                                v_sb[:, g, :, eh * 512:(eh + 1) * 512],
                                start=(g == 0), stop=(g == 3), perf_mode=DR,
                            )
                    pt = pp.tile([128, DIM], bf16, name="pp_t")
                    if split:
                        # tail latency: halve the copy across both engines
                        nc.vector.tensor_copy(out=pt[:, 0:512], in_=po[:, 0:512])
                        nc.scalar.copy(out=pt[:, 512:DIM], in_=po[:, 512:DIM])
                    else:
                        i = ncopy["i"]
                        if i % 4 == 1 or i == 12:
                            nc.scalar.copy(out=pt[:], in_=po)
                        else:
                            nc.vector.tensor_copy(out=pt[:], in_=po)
                    ncopy["i"] += 1
                    nc.sync.dma_start(
                        out=p_out[qc * 128:(qc + 1) * 128, :], in_=pt[:]
                    )

                def rowsum(g4):
                    qs = slice(g4 * 512, (g4 + 1) * 512)
                    psr = pslp.tile([128, 512], f32, name="psl_t")
                    for g in range(4):
                        nc.tensor.matmul(
                            psr[0:1, :],
                            ones_sb[:, :, 0:1],
                            d_sb[:, 2 * g:2 * g + 2, qs],
                            start=(g == 0), stop=(g == 3), perf_mode=DR,
                        )
                    nc.vector.tensor_copy(out=r_sb[:, qs], in_=psr[0:1, :])

                with tc.tile_pool(name="pskan", bufs=2, space="PSUM") as pskan:
                    with tc.tile_pool(name="psv", bufs=2, space="PSUM") as psv:
                        vps = {}

                        def v_mms(kc, gr):
                            if kc not in vps:
                                vps[kc] = psv.tile([128, DIM], f32, name="psv_t")
                            ps = vps[kc]
                            for g in gr:
                                for eh in range(2):
                                    nc.tensor.matmul(
                                        ps[:, eh * 512:(eh + 1) * 512],
                                        xt_sb[:, 2 * g:2 * g + 2, kc * 128:(kc + 1) * 128],
                                        wvt_sb[:, 2 * g:2 * g + 2, eh * 512:(eh + 1) * 512],
                                        start=(g == 0), stop=(g == 3), perf_mode=DR,
                                    )
                            if gr[-1] == 3:
                                if kc % 4 == 3:
                                    nc.scalar.copy(out=v_sb[:, kc // 2, kc % 2, :], in_=vps[kc])
                                else:
                                    nc.vector.tensor_copy(
                                        out=v_sb[:, kc // 2, kc % 2, :], in_=vps[kc]
                                    )
                                del vps[kc]

                        # schedule around DMA arrival: xt keys -> wvt half ->
                        # xt keys 2nd half -> wvt 2nd half -> xt queries
                        kan_group(kanq_sb, 0, 512, cqk_sb[:, 0:1], slice(0, NF))
                        kan_group(kank_sb, 0, 512, cqk_sb[:, 1:2], slice(NF, 32))
                        v_mms(0, [0, 1])
                        v_mms(1, [0, 1])
                        kan_group(kank_sb, 512, 512, cqk_sb[:, 1:2], slice(NF, 32))
                        kan_group(kanq_sb, 512, 512, cqk_sb[:, 0:1], slice(0, NF))
                        v_mms(0, [2, 3])
                        v_mms(1, [2, 3])

                        for mc in range(8):
                            logits_mc(0, mc)
                        for kc in range(2, 8):
                            v_mms(kc, [0, 1, 2, 3])
                        kan_group(kanq_sb, 1024, 512, cqk_sb[:, 0:1], slice(0, NF))
                        kan_group(kanq_sb, 1536, 512, cqk_sb[:, 0:1], slice(0, NF))
                        for mc in range(8):
                            logits_mc(1, mc)

                with tc.tile_pool(name="pso", bufs=3, space="PSUM") as psop:
                    # fine interleave: attn group g with logits group g+2
                    attn_qc(0); logits_mc(2, 0); logits_mc(2, 1)
                    attn_qc(1); logits_mc(2, 2); logits_mc(2, 3)
                    attn_qc(2); logits_mc(2, 4); logits_mc(2, 5)
                    attn_qc(3); logits_mc(2, 6); logits_mc(2, 7)
                    attn_qc(4); rowsum(0)
                    attn_qc(5); logits_mc(3, 0); logits_mc(3, 1)
                    attn_qc(6); logits_mc(3, 2); logits_mc(3, 3)
                    attn_qc(7); logits_mc(3, 4); logits_mc(3, 5)
                    attn_qc(8); logits_mc(3, 6); logits_mc(3, 7)
                    attn_qc(9); rowsum(1)
                    attn_qc(10); attn_qc(11); rowsum(2)
                    attn_qc(12)
                    rowsum(3)
                    nc.sync.dma_start(out=r_out[:], in_=r_sb[:])
                    attn_qc(13, split=True)
                    attn_qc(14, split=True)
                    attn_qc(15, split=True)

    nc.compile()
    return nc


def _get_nc():
    if "nc" not in _cache:
        _cache["nc"] = _build()
    return _cache["nc"]


def kernel(x, basis, Wq, bq, Wk, bk, Wv, bv, _trace=False):
    import ml_dtypes
    from concourse.bass_utils import run_bass_kernel_spmd

    f8 = ml_dtypes.float8_e4m3

    x = np.asarray(x, dtype=np.float32)
    basis = np.asarray(basis, dtype=np.float32)
    Wq = np.asarray(Wq, dtype=np.float32)
    bq = np.asarray(bq, dtype=np.float32)
    Wk = np.asarray(Wk, dtype=np.float32)
    bk = np.asarray(bk, dtype=np.float32)
    Wv = np.asarray(Wv, dtype=np.float32)
    bv = np.asarray(bv, dtype=np.float32)

    SB = np.float32(32.0)
    Bq = (basis @ Wq) * SB            # (16, 1024); exp scale 2^-15 on device
    Bk = (basis @ Wk) * SB
    cq = (basis @ bq) * SB
    ck = (basis @ bk) * SB
    bqk_np = np.zeros((DIM, 32), dtype=np.float32)
    bqk_np[:, 0:NF] = Bq.T
    bqk_np[:, NF:32] = Bk.T
    bqk_np = bqk_np.astype(f8)
    cqk_np = np.stack([cq, ck], axis=1).astype(np.float32)  # (16, 2)
    wvt_np = np.ascontiguousarray(Wv.T).astype(f8)          # (din, e)

    nc = _get_nc()
    in_maps = []
    for c in range(NCORES):
        b, h = c // 2, c % 2
        xtb = x[b].T  # (1024, 2048)
        if h == 1:
            xtb = np.concatenate([xtb[:, MH:], xtb[:, :MH]], axis=1)
        in_maps.append(
            {
                "xt": np.ascontiguousarray(xtb).astype(f8),
                "wvt": wvt_np,
                "bqk": bqk_np,
                "cqk": cqk_np,
            }
        )

    res = run_bass_kernel_spmd(nc, in_maps, list(range(NCORES)), trace=_trace)
    kernel.last_results = res

    # exact colsum-of-v correction on host: c_half = (sum over keys of x) @ Wv.T
    out = np.empty((4, SEQ, DIM), dtype=np.float32)
    for b in range(4):
        c0 = (x[b, :MH, :].sum(axis=0, dtype=np.float64) @ Wv.T.astype(np.float64))
        c1 = (x[b, MH:, :].sum(axis=0, dtype=np.float64) @ Wv.T.astype(np.float64))
        p0 = res.results[2 * b]["p"].astype(np.float32)
        p1 = res.results[2 * b + 1]["p"].astype(np.float32)
        r0 = res.results[2 * b]["r"][0]
        r1 = res.results[2 * b + 1]["r"][0]
        p1 = np.roll(p1, MH, axis=0)
        r1 = np.roll(r1, MH, axis=0)
        num = p0 + p1 + (c0 + c1).astype(np.float32)[None, :]
        den = np.float32(SEQ) + r0 + r1
        out[b] = num / den[:, None] + bv
    return out


# revision 23
# speedup vs baseline: 1.0132x; 1.0062x over previous
"""KAN-attention Trainium2 kernel (8 NeuronCores, SPMD), fp8 DoubleRow version.

Math per batch b:
    q = x Wq^T + bq ; k = x Wk^T + bk ; v = x Wv^T
    kq = q basis^T ; kk = k basis^T            (rank-16)
    out = softmax(kq kk^T / 32) v + bv

Folding: kq = x Bq^T + cq with Bq = basis Wq (host).  Writing e = exp(l)
= 1 + delta, the attention numerator splits as e@v = colsum(v) + delta@v
where colsum(v) is computed EXACTLY on the host (tiny matvec).  The
device only computes p = delta@v and r = rowsum(delta); fp8 quantization
error is then suppressed by |delta| ~ 0.04, so all heavy matmuls run in
fp8e4m3 with DoubleRow (2 contraction rows per PE cell -> 4x fewer PE
cycles than fp32).

Sharding: core c = 2b+h handles batch b and key-half h (1024 of 2048
keys), sequence rotated on host so keys sit at cols 0:1024 of xt.
Host combine: out_b = (p0+p1 + c0+c1) / (2048 + r0+r1) + bv.

Device dataflow (per core), everything fp8 except where noted:
  kan:    psq[16,2048]  = sum_g Bq8[128,2,16].T @ xt[128,2,512]   (DR)
          kanq[16,2048] (bf16) = psq + cq      (ACT, bias)
  v:      psv[128,512]  = sum_g xt[128,2,128k].T @ wvt[128,2,512] (DR)
          v8[128,(g,t),1024] (fp8)             (DVE copy)
  logits: psl[128k,512q] = kank[16,128k].T @ kanq[16,512q]  (bf16, K=16)
  exp:    e[128,512] f32 = Exp(psl * 2^-15)    (ACT)
  delta:  d8[128,(kc),2048] = e - 1 -> fp8     (DVE/Pool)
  attn:   pso[128q,1024e] += d8[128,2,128q].T @ v8[128,2,512e]    (DR)
  rowsum: psr[1,512] += ones[128,2,1].T @ d8[128,2,512q]          (DR)
  out:    p bf16 via engine copy + DMA; r f32.
"""

import os
import sys

sys.path.insert(0, "/opt/trn_rl_repo")

import math

import numpy as np

DIM = 1024
SEQ = 2048
NF = 16
NCORES = 8
MH = 1024  # keys per core

_cache = {}


def _build():
    import concourse.bass as bass
    import concourse.tile as tile
    from concourse import bacc, mybir

    dt = mybir.dt
    f8 = dt.float8e4
    bf16 = dt.bfloat16
    f32 = dt.float32
    DR = mybir.MatmulPerfMode.DoubleRow
    EXPS = 1.0 / 32768.0  # softmax scale 1/32 / (SB*SB) with SB=32

    nc = bacc.Bacc("TRN2", target_bir_lowering=False)

    xt = nc.declare_dram_parameter("xt", [DIM, SEQ], f8, isOutput=False)
    wvt = nc.declare_dram_parameter("wvt", [DIM, DIM], f8, isOutput=False)
    bqk = nc.declare_dram_parameter("bqk", [DIM, 32], f8, isOutput=False)
    cqk = nc.declare_dram_parameter("cqk", [NF, 2], f32, isOutput=False)
    p_out = nc.declare_dram_parameter("p", [SEQ, DIM], bf16, isOutput=True)
    r_out = nc.declare_dram_parameter("r", [1, SEQ], f32, isOutput=True)

    xt_r = xt.rearrange("(o p) l -> p o l", p=128)    # (128, 8, 2048), o=(g,t)
    wvt_r = wvt.rearrange("(o p) e -> p o e", p=128)  # (128, 8, 1024)
    bqk_r = bqk.rearrange("(o p) f -> p o f", p=128)  # (128, 8, 32)

    with tile.TileContext(nc) as tc:
        with tc.tile_pool(name="res", bufs=1) as res:
            xt_sb = res.tile([128, 8, SEQ], f8)
            wvt_sb = res.tile([128, 8, DIM], f8)
            bqk_sb = res.tile([128, 8, 32], f8)
            cqk_sb = res.tile([NF, 2], f32)
            # [128, 2, 16] so the DoubleRow ldweights k-slot stride (16 B)
            # satisfies the ISA step%16==0 constraint; only [:, :, 0:1] is used
            ones_sb = res.tile([128, 2, 16], f8)
            kanq_sb = res.tile([NF, SEQ], bf16)
            kank_sb = res.tile([NF, MH], bf16)
            v_sb = res.tile([128, 4, 2, DIM], f8)     # keys (g,t) on dims 1,2
            d_sb = res.tile([128, 8, SEQ], f8)        # delta^T, dim1 = key chunk
            r_sb = res.tile([1, SEQ], f32)

            nc.vector.memset(ones_sb, 1.0)
            warm_sb = res.tile([1, 8], f32)
            nc.vector.memset(warm_sb, 0.0)
            nc.scalar.activation(
                out=warm_sb, in_=warm_sb,
                func=mybir.ActivationFunctionType.Exp, scale=1.0,
            )

            # input DMAs: key-half of xt + wvt first so the v matmuls can
            # start early; query half streams in behind them
            nc.sync.dma_start(out=bqk_sb[:], in_=bqk_r[:])
            nc.sync.dma_start(out=xt_sb[:, :, 0:512], in_=xt_r[:, :, 0:512])
            nc.sync.dma_start(out=cqk_sb[:], in_=cqk[:])
            nc.sync.dma_start(out=wvt_sb[:, 0:4, :], in_=wvt_r[:, 0:4, :])
            nc.sync.dma_start(out=xt_sb[:, :, 512:MH], in_=xt_r[:, :, 512:MH])
            nc.sync.dma_start(out=wvt_sb[:, 4:8, :], in_=wvt_r[:, 4:8, :])
            nc.sync.dma_start(out=xt_sb[:, :, MH:SEQ], in_=xt_r[:, :, MH:SEQ])

            with (
                tc.tile_pool(name="psl", bufs=2, space="PSUM") as pslp,
                tc.tile_pool(name="ep", bufs=8) as ep,
                tc.tile_pool(name="pp", bufs=4) as pp,
            ):
                ncopy = {"i": 0}

                def kan_group(dst, col0, w, bias, tag):
                    ps = pskan.tile([NF, 512], f32, name="pskan_t")
                    for g in range(4):
                        nc.tensor.matmul(
                            ps[:, 0:w],
                            bqk_sb[:, 2 * g:2 * g + 2, tag],
                            xt_sb[:, 2 * g:2 * g + 2, col0:col0 + w],
                            start=(g == 0), stop=(g == 3), perf_mode=DR,
                        )
                    nc.scalar.activation(
                        out=dst[:, col0:col0 + w], in_=ps[:, 0:w],
                        func=mybir.ActivationFunctionType.Identity,
                        bias=bias, scale=1.0,
                    )

                def logits_mc(qc, mc):
                    qs = slice(qc * 512, (qc + 1) * 512)
                    pl = pslp.tile([128, 512], f32, name="psl_t")
                    nc.tensor.matmul(
                        pl,
                        kank_sb[:, mc * 128:(mc + 1) * 128],
                        kanq_sb[:, qs],
                        start=True, stop=True,
                    )
                    et = ep.tile([128, 512], f32, name="ep_t")
                    nc.scalar.activation(
                        out=et, in_=pl,
                        func=mybir.ActivationFunctionType.Exp,
                        scale=EXPS,
                    )
                    i = qc * 8 + mc
                    if (qc < 2 and i % 4 == 3) or (qc >= 2 and i % 2 == 0):
                        eng = nc.vector
                    else:
                        eng = nc.gpsimd
                    eng.tensor_scalar_sub(
                        out=d_sb[:, mc, qs], in0=et, scalar1=1.0,
                    )

                def attn_qc(qc, split=False):
                    po = psop.tile([128, DIM], f32, name="pso_t")
                    for g in range(4):
                        for eh in range(2):
                            nc.tensor.matmul(
                                po[:, eh * 512:(eh + 1) * 512],
                                d_sb[:, 2 * g:2 * g + 2, qc * 128:(qc + 1) * 128],
                                v_sb[:, g, :, eh * 512:(eh + 1) * 512],
                                start=(g == 0), stop=(g == 3), perf_mode=DR,
                            )
                    pt = pp.tile([128, DIM], bf16, name="pp_t")
                    if split:
                        # tail latency: halve the copy across both engines
                        nc.vector.tensor_copy(out=pt[:, 0:512], in_=po[:, 0:512])
                        nc.scalar.copy(out=pt[:, 512:DIM], in_=po[:, 512:DIM])
                    else:
                        i = ncopy["i"]
                        if i % 4 == 1 or i == 12:
                            nc.scalar.copy(out=pt[:], in_=po)
                        else:
                            nc.vector.tensor_copy(out=pt[:], in_=po)
                    ncopy["i"] += 1
                    nc.sync.dma_start(
                        out=p_out[qc * 128:(qc + 1) * 128, :], in_=pt[:]
                    )

                def rowsum(g4):
                    qs = slice(g4 * 512, (g4 + 1) * 512)
                    psr = pslp.tile([128, 512], f32, name="psl_t")
                    for g in range(4):
                        nc.tensor.matmul(
                            psr[0:1, :],
                            ones_sb[:, :, 0:1],
                            d_sb[:, 2 * g:2 * g + 2, qs],
                            start=(g == 0), stop=(g == 3), perf_mode=DR,
                        )
                    nc.vector.tensor_copy(out=r_sb[:, qs], in_=psr[0:1, :])

                with tc.tile_pool(name="pskan", bufs=2, space="PSUM") as pskan:
                    with tc.tile_pool(name="psv", bufs=2, space="PSUM") as psv:
                        vps = {}

                        def v_mms(kc, gr):
                            if kc not in vps:
                                vps[kc] = psv.tile([128, DIM], f32, name="psv_t")
                            ps = vps[kc]
                            for g in gr:
                                for eh in range(2):
                                    nc.tensor.matmul(
                                        ps[:, eh * 512:(eh + 1) * 512],
                                        xt_sb[:, 2 * g:2 * g + 2, kc * 128:(kc + 1) * 128],
                                        wvt_sb[:, 2 * g:2 * g + 2, eh * 512:(eh + 1) * 512],
                                        start=(g == 0), stop=(g == 3), perf_mode=DR,
                                    )
                            if gr[-1] == 3:
                                nc.vector.tensor_copy(
                                    out=v_sb[:, kc // 2, kc % 2, :], in_=vps[kc]
                                )
                                del vps[kc]

                        # schedule around DMA arrival: xt keys -> wvt half ->
                        # xt keys 2nd half -> wvt 2nd half -> xt queries
                        kan_group(kanq_sb, 0, 512, cqk_sb[:, 0:1], slice(0, NF))
                        kan_group(kank_sb, 0, 512, cqk_sb[:, 1:2], slice(NF, 32))
                        v_mms(0, [0, 1])
                        v_mms(1, [0, 1])
                        kan_group(kank_sb, 512, 512, cqk_sb[:, 1:2], slice(NF, 32))
                        kan_group(kanq_sb, 512, 512, cqk_sb[:, 0:1], slice(0, NF))
                        v_mms(0, [2, 3])
                        v_mms(1, [2, 3])

                        for mc in range(8):
                            logits_mc(0, mc)
                        for kc in range(2, 8):
                            v_mms(kc, [0, 1, 2, 3])
                        kan_group(kanq_sb, 1024, 512, cqk_sb[:, 0:1], slice(0, NF))
                        kan_group(kanq_sb, 1536, 512, cqk_sb[:, 0:1], slice(0, NF))
                        for mc in range(8):
                            logits_mc(1, mc)

                with tc.tile_pool(name="pso", bufs=3, space="PSUM") as psop:
                    # fine interleave: attn group g with logits group g+2
                    attn_qc(0); logits_mc(2, 0); logits_mc(2, 1)
                    attn_qc(1); logits_mc(2, 2); logits_mc(2, 3)
                    attn_qc(2); logits_mc(2, 4); logits_mc(2, 5)
                    attn_qc(3); logits_mc(2, 6); logits_mc(2, 7)
                    attn_qc(4); rowsum(0)
                    attn_qc(5); logits_mc(3, 0); logits_mc(3, 1)
                    attn_qc(6); logits_mc(3, 2); logits_mc(3, 3)
                    attn_qc(7); logits_mc(3, 4); logits_mc(3, 5)
                    attn_qc(8); logits_mc(3, 6); logits_mc(3, 7)
                    attn_qc(9); rowsum(1)
                    attn_qc(10); attn_qc(11); rowsum(2)
                    attn_qc(12)
                    attn_qc(13, split=True)
                    attn_qc(14, split=True)
                    attn_qc(15, split=True)
                    rowsum(3)
                    nc.sync.dma_start(out=r_out[:], in_=r_sb[:])

    nc.compile()
    return nc


def _get_nc():
    if "nc" not in _cache:
        _cache["nc"] = _build()
    return _cache["nc"]


def kernel(x, basis, Wq, bq, Wk, bk, Wv, bv, _trace=False):
    import ml_dtypes
    from concourse.bass_utils import run_bass_kernel_spmd

    f8 = ml_dtypes.float8_e4m3

    x = np.asarray(x, dtype=np.float32)
    basis = np.asarray(basis, dtype=np.float32)
    Wq = np.asarray(Wq, dtype=np.float32)
    bq = np.asarray(bq, dtype=np.float32)
    Wk = np.asarray(Wk, dtype=np.float32)
    bk = np.asarray(bk, dtype=np.float32)
    Wv = np.asarray(Wv, dtype=np.float32)
    bv = np.asarray(bv, dtype=np.float32)

    SB = np.float32(32.0)
    Bq = (basis @ Wq) * SB            # (16, 1024); exp scale 2^-15 on device
    Bk = (basis @ Wk) * SB
    cq = (basis @ bq) * SB
    ck = (basis @ bk) * SB
    bqk_np = np.zeros((DIM, 32), dtype=np.float32)
    bqk_np[:, 0:NF] = Bq.T
    bqk_np[:, NF:32] = Bk.T
    bqk_np = bqk_np.astype(f8)
    cqk_np = np.stack([cq, ck], axis=1).astype(np.float32)  # (16, 2)
    wvt_np = np.ascontiguousarray(Wv.T).astype(f8)          # (din, e)

    nc = _get_nc()
    in_maps = []
    for c in range(NCORES):
        b, h = c // 2, c % 2
        xtb = x[b].T  # (1024, 2048)
        if h == 1:
            xtb = np.concatenate([xtb[:, MH:], xtb[:, :MH]], axis=1)
        in_maps.append(
            {
                "xt": np.ascontiguousarray(xtb).astype(f8),
                "wvt": wvt_np,
                "bqk": bqk_np,
                "cqk": cqk_np,
            }
        )

    res = run_bass_kernel_spmd(nc, in_maps, list(range(NCORES)), trace=_trace)
    kernel.last_results = res

    # exact colsum-of-v correction on host: c_half = (sum over keys of x) @ Wv.T
    out = np.empty((4, SEQ, DIM), dtype=np.float32)
    for b in range(4):
        c0 = (x[b, :MH, :].sum(axis=0, dtype=np.float64) @ Wv.T.astype(np.float64))
        c1 = (x[b, MH:, :].sum(axis=0, dtype=np.float64) @ Wv.T.astype(np.float64))
        p0 = res.results[2 * b]["p"].astype(np.float32)
        p1 = res.results[2 * b + 1]["p"].astype(np.float32)
        r0 = res.results[2 * b]["r"][0]
        r1 = res.results[2 * b + 1]["r"][0]
        p1 = np.roll(p1, MH, axis=0)
        r1 = np.roll(r1, MH, axis=0)
        num = p0 + p1 + (c0 + c1).astype(np.float32)[None, :]
        den = np.float32(SEQ) + r0 + r1
        out[b] = num / den[:, None] + bv
    return out
